# revision 2
# baseline (speedup 1.0000x reference)
"""MoE with adaptive gate on 8 trn2 NeuronCores — v2.1 (fp8 DoubleRow L1).

Data-parallel over batch: each core gets Bs=1024 rows, full weights, no
collectives. Feature-major on-chip layout ([feature, batch]).

Precision plan (tolerance 2e-2; this lands ~3e-3):
  L1 runs as fp8e4 DoubleRow with full operand compensation; the two DR
  K-slots of each instruction carry (x8[dc] (x) w1q[dc]) + (xr8[dc] (x)
  w1d[dc]) from interleaved tensors, plus a pair-pass of x8 (x) w1r:
    w1q=fp8(64 W1), w1d=fp8(W1), w1r=fp8(64 (W1-w1q/64)), xr8=fp8(64 (x-x8))
  accumulating 64*z1 in one psum group.
  Gate logits likewise compensated out of the same xall/gwall tensors:
  pgA = x8@gw8 (64*logits_hi), pgBC = x8@gwr8 + xr8@gw8 (4096*corr), with
  softmax numerator exp(pgA/64+gb) * exp(pgBC/4096).
  L2/L3 run in bf16 (fp8 h2s alone costs 2.1e-2 rel err -- over budget).
  Output stored fp16.

Schedule: gate runs inside the xall DMA fill window (it only needs
xall+gwall), so gates are ready ~13us in; experts then run PE-bound at
w1-trio arrival pace with L2/gating-mul trailing 2 experts behind; L3 is a
clean PE-bound bf16 phase. All psum->sbuf out copies go through DVE to
avoid ACT activation-table swaps (1.28us each).
"""

import sys

sys.path.insert(0, "/opt/trn_rl_repo")

import numpy as np
import ml_dtypes

import concourse.bass as bass
import concourse.tile as tile
from concourse import bacc, mybir
from concourse import bass_utils

B, D, E, H = 8192, 2048, 8, 128
NCORES = 8
Bs = B // NCORES
BT = 512
NBT = Bs // BT
DCH = D // 128            # 16 k-chunks of 128 along D
NPR = DCH // 2            # 8 chunk-pair steps

F32 = mybir.dt.float32
F32R = mybir.dt.float32r
BF16 = mybir.dt.bfloat16
FP16 = mybir.dt.float16
F8 = mybir.dt.float8e4
DR = mybir.MatmulPerfMode.DoubleRow
Silu = mybir.ActivationFunctionType.Silu
Exp = mybir.ActivationFunctionType.Exp

NPF8 = ml_dtypes.float8_e4m3
NPBF = ml_dtypes.bfloat16


def _build_module(reps=1):
    nc = bacc.Bacc("TRN2", target_bir_lowering=False, debug=False,
                   num_devices=NCORES)

    # xall[:, dc, 0, :]=fp8(x), [:, dc, 1, :]=fp8(64*(x-x8)); gwall[:, dc, 0, :]
    # =fp8(4096*gwr) (128-col padded), [:, dc, 1, :]=fp8(64*gw)
    xall = nc.dram_tensor("xall", [128, DCH, 2, Bs], F8,
                          kind="ExternalInput").ap()
    gwall = nc.dram_tensor("gwall", [128, DCH, 2, 128], F8,
                           kind="ExternalInput").ap()
    w1q = nc.dram_tensor("w1q", [E, 128, DCH * H], F8,
                         kind="ExternalInput").ap()
    w1d = nc.dram_tensor("w1d", [E, 128, DCH * H], F8,
                         kind="ExternalInput").ap()
    w1r = nc.dram_tensor("w1r", [E, 128, DCH * H], F8,
                         kind="ExternalInput").ap()
    gb = nc.dram_tensor("gb", [E, 1], F32, kind="ExternalInput").ap()
    b1t = nc.dram_tensor("b1t", [H, E], F32, kind="ExternalInput").ap()
    w2t = nc.dram_tensor("w2t", [H, E * H], BF16, kind="ExternalInput").ap()
    b2t = nc.dram_tensor("b2t", [H, E], F32, kind="ExternalInput").ap()
    # per dc: [128, {q,r}, E, H] with w3q=fp8(16 W3), w3r=fp8(16 (W3-w3q/16))
    w3all = nc.dram_tensor("w3all", [DCH, 128, 2 * E * H], F8,
                           kind="ExternalInput").ap()
    oh = nc.dram_tensor("oh", [E, E * 128], F32R, kind="ExternalInput").ap()
    onesd = nc.dram_tensor("onesd", [E, E], F32R, kind="ExternalInput").ap()
    outT = nc.dram_tensor("outT", [D, Bs], FP16, kind="ExternalOutput").ap()

    with tile.TileContext(nc) as tc:
        with (
            tc.tile_pool(name="persist", bufs=1) as persist,
            tc.tile_pool(name="stream", bufs=2) as stream,
        ):
            # ---- input DMA stream (SP queue, in priority order): gwall
            # first (gate fills the xall window), e0 weights early, small
            # constants after the critical stream head ----
            ones8 = persist.tile([E, 1], F32R, tag="ones8")
            ones1x8 = persist.tile([1, E], F32R, tag="ones1x8")
            gb_sb = persist.tile([E, 1], F32, tag="gb")
            b1_sb = persist.tile([H, E], F32, tag="b1")
            gw_sb = persist.tile([128, DCH, 2, 128], F8, tag="gw")
            x_sb = persist.tile([128, DCH, 2, Bs], F8, tag="xall")
            w1q_sb = [persist.tile([128, DCH, H], F8, tag="w1q", bufs=E,
                                   name=f"w1q_{e}") for e in range(E)]
            w1d_sb = [persist.tile([128, DCH, H], F8, tag="w1d", bufs=E,
                                   name=f"w1d_{e}") for e in range(E)]
            w1r_sb = [persist.tile([128, DCH, H], F8, tag="w1r", bufs=E,
                                   name=f"w1r_{e}") for e in range(E)]

            def _x_dma(j0, j1, slot):
                nc.sync.dma_start(x_sb[:, 2 * j0:2 * j1, slot, :],
                                  xall[:, 2 * j0:2 * j1, slot, :])

            def _w1_dma(e, which):
                dram = {0: w1q, 1: w1d, 2: w1r}[which]
                sb = {0: w1q_sb, 1: w1d_sb, 2: w1r_sb}[which][e]
                nc.sync.dma_start(
                    sb[:], dram[e].rearrange("p (dc h) -> p dc h", h=H))

            # stream: gw8, x8 halves (gate A/BC1 pace), gwr8 between, then
            # xr8 halves + e0 weights (BC2 + e0 pace), then weight trios
            nc.sync.dma_start(gw_sb[:, :, 1, :], gwall[:, :, 1, :])
            _x_dma(0, 1, 0)
            _x_dma(1, 4, 0)
            _x_dma(4, NPR, 0)
            nc.sync.dma_start(gw_sb[:, :, 0, :], gwall[:, :, 0, :])
            nc.sync.dma_start(gb_sb[:], gb[:])
            _x_dma(0, 3, 1)
            _w1_dma(0, 0)
            _x_dma(3, NPR, 1)
            nc.sync.dma_start(b1_sb[:], b1t[:])
            nc.sync.dma_start(ones8[:], onesd[:, 0:1])
            nc.sync.dma_start(ones1x8[:], onesd[0:1, :])
            _w1_dma(0, 1)
            _w1_dma(0, 2)
            for e in range(1, E):
                for w in range(3):
                    _w1_dma(e, w)

            oh_sb = persist.tile([E, E * 128], F32R, tag="oh")
            b2_sb = persist.tile([H, E], F32, tag="b2")
            w2_sb = persist.tile([H, E, H], BF16, tag="w2")
            nc.sync.dma_start(oh_sb[:], oh[:])
            nc.sync.dma_start(b2_sb[:], b2t[:])
            nc.sync.dma_start(w2_sb[:],
                              w2t.rearrange("h (e k) -> h e k", e=E))
            w3_sb = [persist.tile([128, 2, E, H], F8, tag="w3", bufs=DCH,
                                  name=f"w3_{dc}") for dc in range(DCH)]
            for dc in range(DCH):
                nc.sync.dma_start(
                    w3_sb[dc][:],
                    w3all[dc].rearrange("p (t e h) -> p t e h", t=2, h=H))

            gn_sb = persist.tile([E, Bs], F32R, tag="gn")
            h1_sb = [persist.tile([H, Bs], BF16, tag="h1", bufs=E,
                                  name=f"h1_{e}") for e in range(E)]
            h2sA_sb = persist.tile([128, E, Bs], F8, tag="h2sA")
            h2sL_sb = persist.tile([128, E, Bs], F8, tag="h2sL")

            def x8p(j, bs):   # x8 chunk-pair slice [128, 2, BT]
                return x_sb[:, 2 * j:2 * j + 2, 0, bs]

            def xrp(j, bs):   # xr8 chunk-pair slice [128, 2, BT]
                return x_sb[:, 2 * j:2 * j + 2, 1, bs]

            with tc.tile_pool(name="psumA", bufs=1, space="PSUM") as psA:

                # ---- gate logits: run during the xall fill (need only
                # gwall); per-bt sequential, psums ride the acc ring. The
                # softmax's own PE matmuls are deferred into the L1 stream
                # (_gate_z/_gate_pr) so PE never waits on the ACT/DVE chain.
                expT = {}
                recips = {}

                def _gate_logits():
                    pgA = [psA.tile([128, BT], F32, tag="acc", bufs=4,
                                    name=f"pgA{bt}") for bt in range(NBT)]
                    pgBC = [psA.tile([128, BT], F32, tag="acc", bufs=4,
                                     name=f"pgBC{bt}") for bt in range(NBT)]
                    for j in range(NPR):
                        for bt in range(NBT):
                            # A: 64*logits_hi += x8-pair (x) gw8-pair
                            nc.tensor.matmul(
                                pgA[bt][:], gw_sb[:, 2 * j:2 * j + 2, 1, :],
                                x8p(j, bass.ts(bt, BT)), start=(j == 0),
                                stop=(j == NPR - 1), perf_mode=DR)
                    for j in range(NPR):
                        for bt in range(NBT):
                            # BC1: 4096*corr += x8-pair (x) gwr8-pair
                            nc.tensor.matmul(
                                pgBC[bt][:], gw_sb[:, 2 * j:2 * j + 2, 0, :],
                                x8p(j, bass.ts(bt, BT)), start=(j == 0),
                                stop=False, perf_mode=DR)
                    for j in range(NPR):
                        for bt in range(NBT):
                            # BC2: += xr8-pair (x) gw8-pair
                            nc.tensor.matmul(
                                pgBC[bt][:], gw_sb[:, 2 * j:2 * j + 2, 1, :],
                                xrp(j, bass.ts(bt, BT)), start=False,
                                stop=(j == NPR - 1), perf_mode=DR)
                    for bt in range(NBT):
                        expA = stream.tile([E, BT], F32R, tag="expA",
                                           name=f"expA{bt}")
                        nc.scalar.activation(expA[:], pgA[bt][0:E, :], Exp,
                                             bias=gb_sb[:], scale=1.0 / 64.0)
                        expB = stream.tile([E, BT], F32R, tag="expB",
                                           name=f"expB{bt}")
                        nc.scalar.activation(expB[:], pgBC[bt][0:E, :], Exp,
                                             bias=0.0, scale=1.0 / 4096.0)
                        expT[bt] = stream.tile([E, BT], F32R, tag="expT",
                                               name=f"expT{bt}")
                        nc.vector.tensor_mul(expT[bt][:], expA[:], expB[:])

                def _gate_z():
                    for bt in range(NBT):
                        psum_z = psA.tile([1, BT], F32, tag="l2", bufs=2)
                        nc.tensor.matmul(psum_z[:], ones8[:], expT[bt][:],
                                         start=True, stop=True)
                        recips[bt] = stream.tile([1, BT], F32R, tag="recip",
                                                 name=f"recip{bt}")
                        with nc.allow_low_precision(
                                reason="f32r rounding of softmax denom"):
                            nc.vector.reciprocal(recips[bt][:], psum_z[:])

                def _gate_pr():
                    for bt in range(NBT):
                        pr8 = psA.tile([E, BT], F32, tag="l2", bufs=2)
                        nc.tensor.matmul(pr8[:], ones1x8[:], recips[bt][:],
                                         start=True, stop=True)
                        nc.vector.tensor_mul(gn_sb[:, bass.ts(bt, BT)],
                                             expT[bt][:], pr8[:])

                def _l1_expert(e, mid=None):
                    """64*z1 in one psum group via 3 DR pair-passes:
                    x8 (x) w1q, xr8 (x) w1d, x8 (x) w1r; then silu."""
                    ph1 = [psA.tile([H, BT], F32, tag="acc", bufs=4,
                                    name=f"ph1_{e}_{bt}") for bt in range(NBT)]
                    for p, (wt, xf) in enumerate(((w1q_sb, x8p), (w1d_sb, xrp),
                                                  (w1r_sb, x8p))):
                        if p == 2 and mid is not None:
                            mid()
                        for j in range(NPR):
                            for bt in range(NBT):
                                nc.tensor.matmul(
                                    ph1[bt][:], wt[e][:, 2 * j:2 * j + 2, :],
                                    xf(j, bass.ts(bt, BT)),
                                    start=(p == 0 and j == 0),
                                    stop=(p == 2 and j == NPR - 1),
                                    perf_mode=DR)
                    for bt in range(NBT):
                        nc.scalar.activation(h1_sb[e][:, bass.ts(bt, BT)],
                                             ph1[bt][:], Silu,
                                             bias=b1_sb[:, e:e + 1],
                                             scale=1.0 / 64.0)

                def _l2_expert(e):
                    for bt in range(NBT):
                        bs = bass.ts(bt, BT)
                        ph2 = psA.tile([H, BT], F32, tag="l2", bufs=2,
                                       name=f"ph2v_{e}_{bt}")
                        nc.tensor.matmul(ph2[:], w2_sb[:, e, :],
                                         h1_sb[e][:, bs], start=True, stop=True)
                        h2t = stream.tile([H, BT], BF16, tag="h2t", bufs=4)
                        nc.scalar.activation(h2t[:], ph2[:], Silu,
                                             bias=b2_sb[:, e:e + 1], scale=1.0)
                        pgb = psA.tile([128, BT], F32, tag="pgb", bufs=2,
                                       name=f"pgb_{e}_{bt}")
                        nc.tensor.matmul(pgb[:],
                                         oh_sb[:, e * 128:(e + 1) * 128],
                                         gn_sb[:, bs], start=True, stop=True)
                        # v = 16*g*h2: hi fp8 + bf16 value + fp8 residual
                        nc.vector.tensor_mul(h2sA_sb[:, e, bs], h2t[:], pgb[:])
                        h2sV = stream.tile([128, BT], BF16, tag="h2sV",
                                           bufs=4, name=f"h2sV_{e}_{bt}")
                        nc.vector.tensor_mul(h2sV[:], h2t[:], pgb[:])
                        nc.gpsimd.tensor_sub(h2sL_sb[:, e, bs], h2sV[:],
                                             h2sA_sb[:, e, bs])

                # PE order: gate logits fill the xall DMA window, then
                # experts at w1 arrival pace with softmax matmuls and L2
                # (trail-1) slotted between expert blocks; L2[6] lands
                # mid-e7 so only L2[7]'s chain trails the L1 stream.
                _gate_logits()
                _l1_expert(0)
                _gate_z()
                _l1_expert(1)
                _gate_pr()
                _l1_expert(2)
                _l2_expert(0)
                _l2_expert(1)
                for e in range(3, E - 1):
                    _l1_expert(e)
                    _l2_expert(e - 1)
                _l1_expert(E - 1, mid=lambda: _l2_expert(E - 2))
                _l2_expert(E - 1)

            # ---- phase 3: outT[dc] = sum_e W3[e,dc].T @ h2s[e] ----
            with tc.tile_pool(name="psumB", bufs=1, space="PSUM") as psB:
                NEP = E // 2
                PASSES = ((0, 0), (1, 0), (2, 1))  # (rhs sel, w3 slot)
                RHS = {0: h2sA_sb, 1: h2sL_sb, 2: h2sA_sb}

                def _po_drs(po, dc, bs, pairs, first):
                    # 256*y: hi (h2sA x w3q), h2s-residual (h2sL x w3q),
                    # w3-residual (h2sA x w3r)
                    for i in pairs:
                        for p, (rsel, wsl) in enumerate(PASSES):
                            nc.tensor.matmul(
                                po[:], w3_sb[dc][:, wsl, 2 * i:2 * i + 2, :],
                                RHS[rsel][:, 2 * i:2 * i + 2, bs],
                                start=first, stop=(p == 2 and i == NEP - 1),
                                perf_mode=DR)
                            first = False

                def _po_out(po, dc, bt):
                    bs = bass.ts(bt, BT)
                    o_sb = stream.tile([128, BT], FP16, tag="osb", bufs=6,
                                       name=f"osb_{dc}_{bt}")
                    nc.vector.tensor_copy(o_sb[:], po[:])
                    nc.sync.dma_start(outT[dc * 128:(dc + 1) * 128, bs],
                                      o_sb[:])

                HEAD = 4  # first HEAD dc's get runway treatment (8 po banks)
                pos = {}
                # early DRs (expert pairs 0..2) for the head tiles: these only
                # need experts 0-5, so they run while the tail experts' gating
                # is still draining through ACT/DVE/Pool
                for dc in range(HEAD):
                    for bt in range(NBT):
                        po = psB.tile([128, BT], F32, tag="out", bufs=8,
                                      name=f"po_{dc}_{bt}")
                        pos[dc, bt] = po
                        _po_drs(po, dc, bass.ts(bt, BT), range(NEP - 1), True)
                # stop-DRs (pair 3: experts 6,7) + drain for the head tiles
                for dc in range(HEAD):
                    for bt in range(NBT):
                        _po_drs(pos[dc, bt], dc, bass.ts(bt, BT),
                                [NEP - 1], False)
                        _po_out(pos[dc, bt], dc, bt)
                # steady state
                for dc in range(HEAD, DCH):
                    for bt in range(NBT):
                        po = psB.tile([128, BT], F32, tag="out", bufs=8,
                                      name=f"po_{dc}_{bt}")
                        _po_drs(po, dc, bass.ts(bt, BT), range(NEP), True)
                        _po_out(po, dc, bt)

    nc.compile()
    return nc


_MODULE_CACHE = {}


def _get_module(reps=1):
    if reps not in _MODULE_CACHE:
        _MODULE_CACHE[reps] = _build_module(reps)
    return _MODULE_CACHE[reps]


def _q8(a, s=1.0):
    return (np.asarray(a, np.float32) * s).astype(NPF8)


def _prep_in_maps(x, gate_w, gate_b, W1, b1, W2, b2, W3):
    w1qv = _q8(W1, 64.0)
    w1dv = _q8(W1)
    w1rv = _q8(W1 - w1qv.astype(np.float32) / 64.0, 64.0)

    def w1fm(w):  # [E, D, H] -> [E, 128, DCH*H]
        return np.ascontiguousarray(
            w.reshape(E, DCH, 128, H).transpose(0, 2, 1, 3)
        ).reshape(E, 128, DCH * H)

    gw8 = _q8(gate_w, 64.0)
    gwr8 = _q8(gate_w - gw8.astype(np.float32) / 64.0, 4096.0)

    def gfm(g):  # [D, E] -> [128, DCH, 128] zero-padded beyond col E
        out = np.zeros((128, DCH, 128), dtype=g.dtype)
        out[:, :, :E] = g.reshape(DCH, 128, E).transpose(1, 0, 2)
        return out

    gwall = np.ascontiguousarray(
        np.stack([gfm(gwr8), gfm(gw8)], axis=2))          # [128,DCH,2,128]

    w2t = np.ascontiguousarray(
        W2.astype(NPBF).transpose(1, 0, 2)).reshape(H, E * H)
    w3q = _q8(W3, 16.0)
    w3r = _q8(W3 - w3q.astype(np.float32) / 16.0, 16.0)

    def w3fm(w):  # [E, H, D] -> [DCH, 128(h), E, 128(d)]
        return w.reshape(E, H, DCH, 128).transpose(2, 1, 0, 3)

    w3m = np.stack([w3fm(w3q), w3fm(w3r)], axis=2)     # [DCH,H,2,E,128]
    w3m = np.ascontiguousarray(w3m).reshape(DCH, 128, 2 * E * H)

    ohm = np.zeros((E, E * 128), dtype=np.float32)
    for e in range(E):
        ohm[e, e * 128:(e + 1) * 128] = 16.0

    shared = {
        "w1q": w1fm(w1qv), "w1d": w1fm(w1dv), "w1r": w1fm(w1rv),
        "gwall": gwall,
        "gb": np.ascontiguousarray(gate_b.reshape(E, 1)),
        "b1t": np.ascontiguousarray(b1.T), "w2t": w2t,
        "b2t": np.ascontiguousarray(b2.T), "w3all": w3m, "oh": ohm,
        "onesd": np.ones((E, E), dtype=np.float32),
    }
    in_maps = []
    for i in range(NCORES):
        xT = x[i * Bs:(i + 1) * Bs, :].T                  # [D, Bs]
        x8c = _q8(xT)
        xr8c = _q8(xT - x8c.astype(np.float32), 64.0)

        def fm(a):  # [D, Bs] -> [128, DCH, Bs]
            return a.reshape(DCH, 128, Bs).transpose(1, 0, 2)

        xa = np.ascontiguousarray(
            np.stack([fm(x8c), fm(xr8c)], axis=2))        # [128,DCH,2,Bs]
        in_maps.append({"xall": xa, **shared})
    return in_maps


def kernel(x, gate_w, gate_b, W1, b1, W2, b2, W3, b3):
    x = np.asarray(x, dtype=np.float32)
    gate_w = np.asarray(gate_w, dtype=np.float32)
    gate_b = np.asarray(gate_b, dtype=np.float32)
    W1 = np.asarray(W1, dtype=np.float32)
    b1 = np.asarray(b1, dtype=np.float32)
    W2 = np.asarray(W2, dtype=np.float32)
    b2 = np.asarray(b2, dtype=np.float32)
    W3 = np.asarray(W3, dtype=np.float32)
    b3 = np.asarray(b3, dtype=np.float32)

    nc = _get_module(1)
    in_maps = _prep_in_maps(x, gate_w, gate_b, W1, b1, W2, b2, W3)
    try:
        res = bass_utils.run_bass_kernel_spmd(
            nc, in_maps, core_ids=list(range(NCORES)))
    except Exception:
        # transient NRT_EXEC_UNIT_UNRECOVERABLE on the tunneled devices:
        # one retry after a pause clears it
        import time as _time
        _time.sleep(30)
        res = bass_utils.run_bass_kernel_spmd(
            nc, in_maps, core_ids=list(range(NCORES)))

    out = np.empty((B, D), dtype=np.float32)
    for i in range(NCORES):
        out[i * Bs:(i + 1) * Bs, :] = (
            res.results[i]["outT"].astype(np.float32).T * (1.0 / 256.0))

    if np.any(b3):
        # b3 contributes sum_e gates[b,e] * b3[e,d]; zero for this problem,
        # patched on host if ever nonzero.
        logits = x @ gate_w + gate_b
        m = logits.max(axis=1, keepdims=True)
        p = np.exp(logits - m)
        gates = p / p.sum(axis=1, keepdims=True)
        out += gates @ b3
    return out
